# revision 22
# baseline (speedup 1.0000x reference)
"""EdgeConv (gnn_message_passing) Trainium2 Bass kernel — v5.

Computation (reference):
    neigh = x[ind]                                   # [n, k, d] gather
    feat  = [neigh - center, center]                 # [n, k, 2d]
    h     = relu(feat @ W1 + b1) @ W2 + b2           # [n, k, H]
    out   = max over k                               # [n, H]

Algebraic restructuring:
    feat @ W1 = neigh @ W1[:d] + center @ (W1[d:] - W1[:d])
so the kernel consumes slabT = [neighT ; centerT] (feature-major, no
subtraction) against W1' = [[W1[:d]], [W1[d:] - W1[:d]]].  b2 commutes
with the max and is added on the host after the device max-pool.

The irregular gather happens during host-side input staging; the device
streams a fully-built feature-major edge slab (the exact moving-operand
layout the tensor engine wants) and runs a dense pipeline:

  per megablock (24 x 512 points + 1 x 256-point tail per core):
    DMA in  slabT [128, 16*pts] bf16  ([neigh feats ; center feats] x edges)
    PE      matmul1 x2 (W1' stationary) -> p1 [128,1024] PSUM
    ACT     relu+b1 (p1 -> h1 bf16 SBUF, one 1024-col pass)
    PE      matmul2 x2 (W2 stationary), software-pipelined PLAG psum tiles
            behind matmul1 so the relu latency never stalls the PE
    DVE     tensor_reduce(max) over k=16 (one 1024-col pass)
    DMA out mx [128, pts] f32 (transposed; host untransposes)

Engine budget per 512-pt mega: PE 32 matmuls (+32 ldweights), ACT 8 relu,
DVE 8 reduces.  ACT/DVE PSUM passes are the structural floor (PSUM f32
runs 1 elem/cycle); the PE pstate equilibrates against them.

Data-parallel over points: 8 cores x 12500 points (padded to 12544).
"""

import os
import sys

for _p in ("/opt/trn_rl_repo",):
    if _p not in sys.path and os.path.isdir(_p):
        sys.path.insert(0, _p)

import numpy as np
import ml_dtypes

BF16 = ml_dtypes.bfloat16

# problem constants (hardcoded per harness contract)
N, D, K, H = 100000, 64, 16, 128
NCORES = 8
NP = 12500            # points per core
MEGA = 512            # points per full megablock
MSIZES = [MEGA] * 24 + [256]        # megablock sizes (sum = NPP)
NPP = sum(MSIZES)     # padded points per core (12544)
EDGES = MEGA * K      # 8192 edges per full megablock
GCOLS = 512           # matmul free-dim tile (one PSUM bank)
PLAG = 3              # matmul2 runs this many psum pair-tiles behind matmul1


class Cfg:
    def __init__(self):
        self.n = N
        self.np = NP
        self.npp = NPP
        self.msizes = list(MSIZES)


def build_program(cfg: Cfg, debug=False):
    import concourse.bacc as bacc
    import concourse.bass as bass
    import concourse.tile as tile
    from concourse import mybir

    f32 = mybir.dt.float32
    bf16 = mybir.dt.bfloat16

    nc = bacc.Bacc("TRN2", target_bir_lowering=False, debug=debug)

    nbt = nc.dram_tensor("nbt", (2 * D, cfg.npp * K), bf16,
                         kind="ExternalInput")
    w1 = nc.dram_tensor("w1", (2 * D, H), bf16, kind="ExternalInput")
    w2 = nc.dram_tensor("w2", (H, H), bf16, kind="ExternalInput")
    b1 = nc.dram_tensor("b1", (H, 1), f32, kind="ExternalInput")
    out2 = nc.dram_tensor("out2", (H, cfg.npp), f32, kind="ExternalOutput")

    with tile.TileContext(nc) as tc:
        with (
            tc.tile_pool(name="const", bufs=1) as constp,
            tc.tile_pool(name="slab", bufs=4) as slabp,
            tc.tile_pool(name="h1", bufs=PLAG + 2) as h1p,
            tc.tile_pool(name="mx", bufs=3) as mxp,
            tc.tile_pool(name="ps1", bufs=2, space="PSUM") as ps1p,
            tc.tile_pool(name="ps2", bufs=2, space="PSUM") as ps2p,
        ):
            # weights ride the Activation HWDGE queue so they load in
            # parallel with the first slab chunks on the sync queue
            w1s = constp.tile([2 * D, H], bf16)
            nc.scalar.dma_start(w1s[:], w1[:, :])
            w2s = constp.tile([H, H], bf16)
            nc.scalar.dma_start(w2s[:], w2[:, :])
            b1s = constp.tile([H, 1], f32)
            nc.scalar.dma_start(b1s[:], b1[:, :])

            p_off = 0
            for msz in cfg.msizes:
                medges = msz * K
                np2 = medges // (2 * GCOLS)   # psum pair-tiles this mega
                e_off = p_off * K

                slab = slabp.tile([128, EDGES], bf16)
                # split the stream so matmuls on the first chunk can start
                # while the rest is still in flight (subtile deps); the lead
                # chunk is one psum tile's worth so slot 0 unblocks fastest
                cuts = [0, 2 * GCOLS, medges - (medges - 2 * GCOLS) // 2, medges]
                for a, b in zip(cuts, cuts[1:]):
                    if b > a:
                        nc.sync.dma_start(
                            slab[:, a:b], nbt[:, e_off + a:e_off + b])

                mx = mxp.tile([H, MEGA], f32)
                h1s = [None] * np2

                def do_m1(t):
                    p1 = ps1p.tile([H, 2 * GCOLS], f32)
                    for j in range(2):
                        nc.tensor.matmul(
                            p1[:, j * GCOLS:(j + 1) * GCOLS], lhsT=w1s[:],
                            rhs=slab[:, (2 * t + j) * GCOLS:
                                     (2 * t + j + 1) * GCOLS],
                            start=True, stop=True,
                        )
                    h1 = h1p.tile([H, 2 * GCOLS], bf16)
                    nc.scalar.activation(
                        h1[:], p1[:], mybir.ActivationFunctionType.Relu,
                        bias=b1s[:], scale=1.0,
                    )
                    h1s[t] = h1

                def do_m2(t):
                    p2 = ps2p.tile([H, 2 * GCOLS], f32)
                    for j in range(2):
                        nc.tensor.matmul(
                            p2[:, j * GCOLS:(j + 1) * GCOLS], lhsT=w2s[:],
                            rhs=h1s[t][:, j * GCOLS:(j + 1) * GCOLS],
                            start=True, stop=True,
                        )
                    nc.vector.tensor_reduce(
                        out=mx[:, t * (2 * GCOLS // K):
                               (t + 1) * (2 * GCOLS // K)],
                        in_=p2[:].rearrange("p (a b) -> p a b", b=K),
                        axis=mybir.AxisListType.X,
                        op=mybir.AluOpType.max,
                    )

                # software-pipelined: matmul1 pair-tiles run PLAG ahead of
                # matmul2 pair-tiles so relu latency is off the PE path
                lag = min(PLAG, np2)
                for t in range(np2):
                    do_m1(t)
                    if t >= lag:
                        do_m2(t - lag)
                for t in range(np2 - lag, np2):
                    do_m2(t)

                nc.sync.dma_start(
                    out2[:, p_off:p_off + msz], mx[:, :msz]
                )
                p_off += msz

    nc.compile()
    return nc


def host_prep(cfg: Cfg, x, W1, b1, W2, b2):
    """Shared (core-independent) input prep."""
    xbT = np.ascontiguousarray(x.astype(BF16).T)     # [D, N] feature-major
    what = np.vstack([W1[:D], W1[D:] - W1[:D]]).astype(BF16)
    w2b = W2.astype(BF16)
    b1c = np.ascontiguousarray(b1.astype(np.float32).reshape(H, 1))
    b2c = np.ascontiguousarray(b2.astype(np.float32).reshape(H, 1))
    return xbT, what, w2b, b1c, b2c


def core_inputs(cfg: Cfg, xbT, what, w2b, b1c, b2c, ind32, lo, hi):
    """Build one core's input map for its point range [lo, hi)."""
    indc = np.zeros((cfg.npp, K), np.int32)
    indc[:hi - lo] = ind32[lo:hi]
    flat = indc.reshape(-1)                          # edge e = 16*p + k
    nbt = np.empty((2 * D, cfg.npp * K), BF16)
    nbt[:D] = xbT[:, flat]                           # neighbor features
    xc = np.zeros((D, cfg.npp), BF16)
    xc[:, :hi - lo] = xbT[:, lo:hi]
    # center features, replicated over each point's K edge columns
    nbt[D:] = np.repeat(xc, K, axis=1)
    return {
        "nbt": nbt,
        "w1": what, "w2": w2b, "b1": b1c,
    }


_NC_CACHE = {}


def kernel(x, ind, W1, b1, W2, b2):
    from concourse import bass_utils

    cfg = Cfg()
    key = (cfg.n, cfg.np, cfg.npp)
    if key not in _NC_CACHE:
        _NC_CACHE[key] = build_program(cfg)
    nc = _NC_CACHE[key]

    x = np.asarray(x, np.float32)
    ind32 = np.asarray(ind).astype(np.int32)
    xbT, what, w2b, b1c, b2c = host_prep(cfg, x, np.asarray(W1, np.float32),
                                         np.asarray(b1, np.float32),
                                         np.asarray(W2, np.float32),
                                         np.asarray(b2, np.float32))
    in_maps = []
    for c in range(NCORES):
        lo = c * NP
        hi = min(lo + NP, N)
        in_maps.append(core_inputs(cfg, xbT, what, w2b, b1c, b2c, ind32, lo, hi))

    res = bass_utils.run_bass_kernel_spmd(nc, in_maps, core_ids=list(range(NCORES)))
    b2f = np.asarray(b2, np.float32).reshape(1, H)
    out = np.empty((N, H), np.float32)
    for c in range(NCORES):
        lo = c * NP
        hi = min(lo + NP, N)
        out[lo:hi] = res.results[c]["out2"].T[:hi - lo] + b2f
    return out


# revision 23
# speedup vs baseline: 1.0106x; 1.0106x over previous
"""EdgeConv (gnn_message_passing) Trainium2 Bass kernel — v5.

Computation (reference):
    neigh = x[ind]                                   # [n, k, d] gather
    feat  = [neigh - center, center]                 # [n, k, 2d]
    h     = relu(feat @ W1 + b1) @ W2 + b2           # [n, k, H]
    out   = max over k                               # [n, H]

Algebraic restructuring:
    feat @ W1 = neigh @ W1[:d] + center @ (W1[d:] - W1[:d])
so the kernel consumes slabT = [neighT ; centerT] (feature-major, no
subtraction) against W1' = [[W1[:d]], [W1[d:] - W1[:d]]].  b2 commutes
with the max and is added on the host after the device max-pool.

The irregular gather happens during host-side input staging; the device
streams a fully-built feature-major edge slab (the exact moving-operand
layout the tensor engine wants) and runs a dense pipeline:

  per megablock (24 x 512 points + 1 x 256-point tail per core):
    DMA in  slabT [128, 16*pts] bf16  ([neigh feats ; center feats] x edges)
    PE      matmul1 x2 (W1' stationary) -> p1 [128,1024] PSUM
    ACT     relu+b1 (p1 -> h1 bf16 SBUF, one 1024-col pass)
    PE      matmul2 x2 (W2 stationary), software-pipelined PLAG psum tiles
            behind matmul1 so the relu latency never stalls the PE
    DVE     tensor_reduce(max) over k=16 (one 1024-col pass)
    DMA out mx [128, pts] f32 (transposed; host untransposes)

Engine budget per 512-pt mega: PE 32 matmuls (+32 ldweights), ACT 8 relu,
DVE 8 reduces.  ACT/DVE PSUM passes are the structural floor (PSUM f32
runs 1 elem/cycle); the PE pstate equilibrates against them.

Data-parallel over points: 8 cores x 12500 points (padded to 12544).
"""

import os
import sys

for _p in ("/opt/trn_rl_repo",):
    if _p not in sys.path and os.path.isdir(_p):
        sys.path.insert(0, _p)

import numpy as np
import ml_dtypes

BF16 = ml_dtypes.bfloat16

# problem constants (hardcoded per harness contract)
N, D, K, H = 100000, 64, 16, 128
NCORES = 8
NP = 12500            # points per core
MEGA = 512            # points per full megablock
MSIZES = [MEGA] * 24 + [256]        # megablock sizes (sum = NPP)
NPP = sum(MSIZES)     # padded points per core (12544)
EDGES = MEGA * K      # 8192 edges per full megablock
GCOLS = 512           # matmul free-dim tile (one PSUM bank)
PLAG = 3              # matmul2 runs this many psum pair-tiles behind matmul1


class Cfg:
    def __init__(self):
        self.n = N
        self.np = NP
        self.npp = NPP
        self.msizes = list(MSIZES)


def build_program(cfg: Cfg, debug=False):
    import concourse.bacc as bacc
    import concourse.bass as bass
    import concourse.tile as tile
    from concourse import mybir

    f32 = mybir.dt.float32
    bf16 = mybir.dt.bfloat16

    nc = bacc.Bacc("TRN2", target_bir_lowering=False, debug=debug)

    nbt = nc.dram_tensor("nbt", (2 * D, cfg.npp * K), bf16,
                         kind="ExternalInput")
    w1 = nc.dram_tensor("w1", (2 * D, H), bf16, kind="ExternalInput")
    w2 = nc.dram_tensor("w2", (H, H), bf16, kind="ExternalInput")
    b1 = nc.dram_tensor("b1", (H, 1), f32, kind="ExternalInput")
    out2 = nc.dram_tensor("out2", (H, cfg.npp), f32, kind="ExternalOutput")

    with tile.TileContext(nc) as tc:
        with (
            tc.tile_pool(name="const", bufs=1) as constp,
            tc.tile_pool(name="slab", bufs=4) as slabp,
            tc.tile_pool(name="h1", bufs=PLAG + 2) as h1p,
            tc.tile_pool(name="mx", bufs=3) as mxp,
            tc.tile_pool(name="ps1", bufs=2, space="PSUM") as ps1p,
            tc.tile_pool(name="ps2", bufs=2, space="PSUM") as ps2p,
        ):
            w1s = constp.tile([2 * D, H], bf16)
            nc.sync.dma_start(w1s[:], w1[:, :])
            w2s = constp.tile([H, H], bf16)
            nc.sync.dma_start(w2s[:], w2[:, :])
            b1s = constp.tile([H, 1], f32)
            nc.sync.dma_start(b1s[:], b1[:, :])

            p_off = 0
            for msz in cfg.msizes:
                medges = msz * K
                np2 = medges // (2 * GCOLS)   # psum pair-tiles this mega
                e_off = p_off * K

                slab = slabp.tile([128, EDGES], bf16)
                # split the stream so matmuls on the first chunk can start
                # while the rest is still in flight (subtile deps); the lead
                # chunk is one psum tile's worth so slot 0 unblocks fastest
                cuts = [0, 2 * GCOLS, medges - (medges - 2 * GCOLS) // 2, medges]
                for a, b in zip(cuts, cuts[1:]):
                    if b > a:
                        nc.sync.dma_start(
                            slab[:, a:b], nbt[:, e_off + a:e_off + b])

                mx = mxp.tile([H, MEGA], f32)
                h1s = [None] * np2

                def do_m1(t):
                    p1 = ps1p.tile([H, 2 * GCOLS], f32)
                    for j in range(2):
                        nc.tensor.matmul(
                            p1[:, j * GCOLS:(j + 1) * GCOLS], lhsT=w1s[:],
                            rhs=slab[:, (2 * t + j) * GCOLS:
                                     (2 * t + j + 1) * GCOLS],
                            start=True, stop=True,
                        )
                    h1 = h1p.tile([H, 2 * GCOLS], bf16)
                    nc.scalar.activation(
                        h1[:], p1[:], mybir.ActivationFunctionType.Relu,
                        bias=b1s[:], scale=1.0,
                    )
                    h1s[t] = h1

                def do_m2(t):
                    p2 = ps2p.tile([H, 2 * GCOLS], f32)
                    for j in range(2):
                        nc.tensor.matmul(
                            p2[:, j * GCOLS:(j + 1) * GCOLS], lhsT=w2s[:],
                            rhs=h1s[t][:, j * GCOLS:(j + 1) * GCOLS],
                            start=True, stop=True,
                        )
                    nc.vector.tensor_reduce(
                        out=mx[:, t * (2 * GCOLS // K):
                               (t + 1) * (2 * GCOLS // K)],
                        in_=p2[:].rearrange("p (a b) -> p a b", b=K),
                        axis=mybir.AxisListType.X,
                        op=mybir.AluOpType.max,
                    )

                # software-pipelined: matmul1 pair-tiles run PLAG ahead of
                # matmul2 pair-tiles so relu latency is off the PE path
                lag = min(PLAG, np2)
                for t in range(np2):
                    do_m1(t)
                    if t >= lag:
                        do_m2(t - lag)
                for t in range(np2 - lag, np2):
                    do_m2(t)

                nc.sync.dma_start(
                    out2[:, p_off:p_off + msz], mx[:, :msz]
                )
                p_off += msz

    nc.compile()
    return nc


def host_prep(cfg: Cfg, x, W1, b1, W2, b2):
    """Shared (core-independent) input prep."""
    xbT = np.ascontiguousarray(x.astype(BF16).T)     # [D, N] feature-major
    what = np.vstack([W1[:D], W1[D:] - W1[:D]]).astype(BF16)
    w2b = W2.astype(BF16)
    b1c = np.ascontiguousarray(b1.astype(np.float32).reshape(H, 1))
    b2c = np.ascontiguousarray(b2.astype(np.float32).reshape(H, 1))
    return xbT, what, w2b, b1c, b2c


def core_inputs(cfg: Cfg, xbT, what, w2b, b1c, b2c, ind32, lo, hi):
    """Build one core's input map for its point range [lo, hi)."""
    indc = np.zeros((cfg.npp, K), np.int32)
    indc[:hi - lo] = ind32[lo:hi]
    flat = indc.reshape(-1)                          # edge e = 16*p + k
    nbt = np.empty((2 * D, cfg.npp * K), BF16)
    nbt[:D] = xbT[:, flat]                           # neighbor features
    xc = np.zeros((D, cfg.npp), BF16)
    xc[:, :hi - lo] = xbT[:, lo:hi]
    # center features, replicated over each point's K edge columns
    nbt[D:] = np.repeat(xc, K, axis=1)
    return {
        "nbt": nbt,
        "w1": what, "w2": w2b, "b1": b1c,
    }


_NC_CACHE = {}


def kernel(x, ind, W1, b1, W2, b2):
    from concourse import bass_utils

    cfg = Cfg()
    key = (cfg.n, cfg.np, cfg.npp)
    if key not in _NC_CACHE:
        _NC_CACHE[key] = build_program(cfg)
    nc = _NC_CACHE[key]

    x = np.asarray(x, np.float32)
    ind32 = np.asarray(ind).astype(np.int32)
    xbT, what, w2b, b1c, b2c = host_prep(cfg, x, np.asarray(W1, np.float32),
                                         np.asarray(b1, np.float32),
                                         np.asarray(W2, np.float32),
                                         np.asarray(b2, np.float32))
    in_maps = []
    for c in range(NCORES):
        lo = c * NP
        hi = min(lo + NP, N)
        in_maps.append(core_inputs(cfg, xbT, what, w2b, b1c, b2c, ind32, lo, hi))

    res = bass_utils.run_bass_kernel_spmd(nc, in_maps, core_ids=list(range(NCORES)))
    b2f = np.asarray(b2, np.float32).reshape(1, H)
    out = np.empty((N, H), np.float32)
    for c in range(NCORES):
        lo = c * NP
        hi = min(lo + NP, N)
        out[lo:hi] = res.results[c]["out2"].T[:hi - lo] + b2f
    return out
